# revision 1
# baseline (speedup 1.0000x reference)
"""EquivariantProjectorViaSchur — TRN2 Bass kernel (8 NeuronCores, SPMD).

Math (per 64x64 channel block B of W):
    V   = U_y^T B U_x
    P   = A o V + Bc o V[sig_r][:, sig_c]     (= mask + gather-symmetrize-scatter)
    out = U_y P U_x^T
The masked symmetrization is fused into the PE matmuls via the k-group
structure of the Schur mask (8 rotation groups of 6, 2 parity groups of 8):
    Z[:, o in g]   = (s_g XG_g) @ T1T[:, o in g] + XJ_g @ T1s[:, o in g]
    XG_g = U_x diag(a_g) U_x^T        (symmetric; s_g = 1/2 rot, 1 diag)
    XJ_g[k',q] = 1/2 sum_{k in g} pi_k U_x[k',k] U_x[q, k^1]
    T1T  = (U_y^T B)^T  (produced directly by W-stationary matmuls)
    T1s[q,o] = pi_o * T1T[q, o^1]     (one DVE copy + one negate, strided APs)
    out  = kron(I2, U_y^T)-contraction of Z^T  (PE transpose + matmul)
Sharding: c_in block-columns — core i owns W[:, i*768:(i+1)*768]; the tiny
U/mask-derived factor matrices are replicated (precomputed host-side).
"""
import contextlib
import time

import numpy as np

import concourse.bass as bass
import concourse.tile as tile
import concourse.mybir as mybir
from concourse.tile import ScopedClock

F32 = mybir.dt.float32
F32R = mybir.dt.float32r

O = 64
NSTR_CH = 24          # 128-row stripes per b-chunk
NCH = 2               # b chunks of 48 blocks
NQ = 3                # c quads (4 c-blocks = 256 cols each)
NCORE = 8
CSH = 768             # columns per core shard


# ---------------------------------------------------------------------------
# workarounds for this toolchain
# ---------------------------------------------------------------------------
def _patched_drain_and_barrier(self, tick_clock, wait_clock):
    # this walrus build rejects >1 sem-wait on a Drain: split the tail waits
    drain_inst = self.nc.sync.drain()
    wait_clock.add_sem_waits(drain_inst.ins,
                             ScopedClock({None: tick_clock.global_clock}))
    si = drain_inst.ins.sync_info
    waits = list(si.on_wait) if si is not None else []
    if len(waits) > 1:
        drain_inst.ins.sync_info = mybir.SyncInfo(
            on_wait=waits[:1], on_update=list(si.on_update))
        for i in range(1, len(waits)):
            d2 = self.nc.sync.drain()
            d2.ins.sync_info = mybir.SyncInfo(on_wait=[waits[i]], on_update=[])
    self.nc.all_engine_barrier()
    assert self.sems is not None
    popped = self.nc._tile_sem_poison_stack.pop()
    assert popped is self._sem_poison
    self.nc.clear_and_free_semaphores(list(self.sems.allocated().values()))
    self.nc.all_engine_barrier()


tile.TileContext._drain_and_barrier = _patched_drain_and_barrier


def cap_sync_waits(nc):
    """walrus codegen allows only 1 sem-wait per instruction struct here;
    carry the excess on NoOps inserted just before (same engine/point)."""
    for f in nc.m.functions:
        for blk in f.blocks:
            insts = list(blk.instructions)
            out = []
            ctr = 0
            for ins in insts:
                si = ins.sync_info
                waits = list(si.on_wait) if si is not None else []
                if len(waits) > 1:
                    for i in range(len(waits) - 1):
                        n = mybir.InstNoOp(name=f"{ins.name}_w{ctr}",
                                           ins=[], outs=[])
                        ctr += 1
                        n.engine = ins.engine
                        n.sync_info = mybir.SyncInfo(on_wait=[waits[i]],
                                                     on_update=[])
                        out.append(n)
                    ins.sync_info = mybir.SyncInfo(
                        on_wait=waits[-1:], on_update=list(si.on_update))
                out.append(ins)
            blk.instructions = out


_LDW_PATCHED = False


def enable_ldw_opt():
    """--enable-ldw-opt=true: skip redundant LDWEIGHTS for runs of matmuls
    sharing the same stationary operand."""
    global _LDW_PATCHED
    if _LDW_PATCHED:
        return
    _LDW_PATCHED = True
    import concourse.bass_utils as bu
    orig = bu.bir_verify_and_optimise

    def patched(tmpdir, inp="bir.json", outp="file.neff", arch=None, *,
                dve_root=None):
        real_run = bu.run_command

        def run_hook(argv, **kw):
            argv = ["--enable-ldw-opt=true" if a == "--enable-ldw-opt=false"
                    else a for a in argv]
            return real_run(argv, **kw)
        bu.run_command = run_hook
        try:
            return orig(tmpdir, inp, outp, arch, dve_root=dve_root)
        finally:
            bu.run_command = real_run
    bu.bir_verify_and_optimise = patched


# ---------------------------------------------------------------------------
# host-side precompute of the replicated factor matrices
# ---------------------------------------------------------------------------
def host_precompute(U_y, U_x, mask, block_rows, block_cols):
    rows = np.asarray(block_rows); cols = np.asarray(block_cols)
    mask = np.asarray(mask)
    U_y64 = np.asarray(U_y, np.float64); U_x64 = np.asarray(U_x, np.float64)
    r_rot = set(int(x) for x in rows.tolist())
    nqd = len(rows) // 4
    for t in range(nqd):
        r = rows[4 * t:4 * t + 4]; c = cols[4 * t:4 * t + 4]
        assert mask[r, c].all()
        assert r[0] == r[1] and r[2] == r[3] and r[2] == r[0] + 1 and r[0] % 2 == 0
        assert c[0] == c[2] and c[1] == c[3] and c[1] == c[0] + 1 and c[0] % 2 == 0
    groups, seen = [], np.zeros(O, bool)
    for k in range(O):
        if seen[k]:
            continue
        mem = np.where(mask[k] > 0)[0]
        assert (mask[np.ix_(mem, mem)] > 0).all()
        for m in mem:
            seen[m] = True
        groups.append(mem)
    pi = np.where(np.arange(O) % 2 == 0, 1.0, -1.0)
    eye2 = np.eye(2)
    mats, ginfo = [], []
    mats.append(np.kron(eye2, U_y64).astype(np.float32))    # 0: LY (S1 moving)
    mats.append(np.kron(eye2, U_y64.T).astype(np.float32))  # 1: LS4
    mats.append(np.eye(128, dtype=np.float32))              # 2: identity
    for mem in groups:
        is_R = int(mem[0]) in r_rot
        s = 0.5 if is_R else 1.0
        a = np.zeros(O); a[mem] = 1.0
        XG = s * (U_x64 @ np.diag(a) @ U_x64.T)
        gi_idx = len(mats); mats.append(np.kron(eye2, XG).astype(np.float32))
        ji_idx = None
        if is_R:
            assert len(mem) == (mem[-1] - mem[0] + 1), "rot group not contiguous"
            XJ = np.zeros((O, O))
            for k in mem:
                XJ += 0.5 * pi[k] * np.outer(U_x64[:, k], U_x64[:, k ^ 1])
            ji_idx = len(mats); mats.append(np.kron(eye2, XJ.T).astype(np.float32))
        else:
            st = int(mem[0])
            assert all(int(m) == st + 2 * i for i, m in enumerate(mem)), \
                "diag group not stride-2"
        ginfo.append(dict(mem=[int(x) for x in mem], is_R=is_R,
                          gi=gi_idx, ji=ji_idx))
    const = np.concatenate(mats, axis=1)
    return np.ascontiguousarray(const), ginfo


class _EvacBalancer:
    """Greedy ACT/DVE assignment for PSUM->SBUF copies."""
    def __init__(self, nc):
        self.nc = nc
        self.t_act = 0.0
        self.t_dve = 0.0

    def copy(self, dst, src):
        fd = src.free_size()
        c_act = (172.0 + fd) / 1.2
        c_dve = (120.0 + fd) / 0.96
        if self.t_act + c_act <= self.t_dve + c_dve:
            self.t_act += c_act
            return self.nc.scalar.copy(dst, src)
        else:
            self.t_dve += c_dve
            return self.nc.vector.tensor_copy(dst, src)


# ---------------------------------------------------------------------------
# device kernel (one program, SPMD over 8 cores)
# ---------------------------------------------------------------------------
def build_kernel(n_const_mats, ginfo):
    nc = bass.Bass("TRN2", target_bir_lowering=False, debug=False,
                   num_devices=1)
    w = nc.dram_tensor("w", [6144, CSH], F32R, kind="ExternalInput").ap()
    cst = nc.dram_tensor("cst", [128, n_const_mats * 128], F32R,
                         kind="ExternalInput").ap()
    out = nc.dram_tensor("out", [NQ, 12, 128, 1024], F32,
                         kind="ExternalOutput").ap()

    with tile.TileContext(nc) as tc:
        ctx = contextlib.ExitStack()
        with ctx:
            ev = _EvacBalancer(nc)
            csb_p = ctx.enter_context(tc.tile_pool(name="cst", bufs=1))
            wch_p = ctx.enter_context(tc.tile_pool(name="wch", bufs=10))
            t1T_p = ctx.enter_context(tc.tile_pool(name="t1T", bufs=1))
            t1s_p = ctx.enter_context(tc.tile_pool(name="t1s", bufs=1))
            zsb_p = ctx.enter_context(tc.tile_pool(name="zsb", bufs=1))
            ztsb_p = ctx.enter_context(tc.tile_pool(name="ztsb", bufs=1))
            osb_p = ctx.enter_context(tc.tile_pool(name="osb", bufs=6))
            ps_1b = ctx.enter_context(
                tc.tile_pool(name="ps_1b", bufs=8, space="PSUM"))

            csb = csb_p.tile([128, n_const_mats * 128], F32R)
            nc.sync.dma_start(csb[:], cst[:])

            def cmat(i):
                return csb[:, i * 128:(i + 1) * 128]

            LY, LS4 = cmat(0), cmat(1)
            ident = cmat(2)

            for ch in range(NCH):
                # phase A: W-stationary S1' emits T1^T pieces directly
                t1Tq = [[t1T_p.tile([128, NSTR_CH * 128], F32R,
                                    tag=f"t1T{q}_{cp}",
                                    name=f"t1T_{ch}_{q}_{cp}")
                         for cp in range(2)] for q in range(NQ)]
                for sg in range(0, NSTR_CH, 4):
                    grp = []
                    for k4 in range(4):
                        s = sg + k4
                        t = wch_p.tile([128, CSH], F32R, tag="w",
                                       name=f"w_{ch}_{s}")
                        r0 = (ch * NSTR_CH + s) * 128
                        nc.sync.dma_start(t[:], w[r0:r0 + 128, :])
                        grp.append(t)
                    for q in range(NQ):
                        for cp in range(2):
                            pb = ps_1b.tile([128, 512], F32, tag="pb",
                                            name="pb")
                            for k in range(4):
                                lhsT = grp[k][:, (q * 2 + cp) * 128:
                                              (q * 2 + cp + 1) * 128]
                                nc.tensor.matmul(
                                    pb[:, k * 128:(k + 1) * 128], lhsT, LY)
                            ev.copy(t1Tq[q][cp][:, sg * 128:(sg + 4) * 128],
                                    pb[:])
                # phase B per quad: sigma prep, fused group matmuls, transpose,
                # final contraction, store
                for q in range(NQ):
                    t1T = t1Tq[q]
                    t1s = [t1s_p.tile([128, 48 * 48], F32R, tag=f"t1s{cp}",
                                      name=f"t1s_{ch}_{q}_{cp}")
                           for cp in range(2)]
                    for cp in range(2):
                        tv = t1T[cp][:].rearrange("p (b o) -> p b o", o=64)
                        sv = t1s[cp][:].rearrange("p (b o) -> p b o", o=48)
                        nc.vector.tensor_copy(sv[:, :, 0:48:2],
                                              tv[:, :, 1:48:2])
                        nc.vector.tensor_scalar_mul(sv[:, :, 1:48:2],
                                                    tv[:, :, 0:48:2], -1.0)
                    zsb = [zsb_p.tile([128, NSTR_CH * 128], F32R,
                                      tag=f"z{cp}", name=f"z_{ch}_{q}_{cp}")
                           for cp in range(2)]
                    tvs = [t1T[cp][:].rearrange("p (b o) -> p b o", o=64)
                           for cp in range(2)]
                    svs = [t1s[cp][:].rearrange("p (b o) -> p b o", o=48)
                           for cp in range(2)]
                    zvs = [zsb[cp][:].rearrange("p (b o) -> p b o", o=64)
                           for cp in range(2)]
                    for g in ginfo:
                        mem = g["mem"]
                        if g["is_R"]:
                            zps = []
                            for cp in range(2):
                                zp = ps_1b.tile([128, 48 * 6], F32, tag="pb",
                                                name="zp")
                                nc.tensor.matmul(
                                    zp[:], cmat(g["gi"]),
                                    tvs[cp][:, :, mem[0]:mem[0] + 6],
                                    start=True, stop=False)
                                zps.append(zp)
                            for cp in range(2):
                                nc.tensor.matmul(
                                    zps[cp][:], cmat(g["ji"]),
                                    svs[cp][:, :, mem[0]:mem[0] + 6],
                                    start=False, stop=True)
                                dst = zvs[cp][:, :, mem[0]:mem[0] + 6]
                                ev.copy(dst, zps[cp][:].rearrange(
                                    "p (b o) -> p b o", o=6))
                        else:
                            st = mem[0]
                            for cp in range(2):
                                zp = ps_1b.tile([128, 48 * 8], F32, tag="pb",
                                                name="zp")
                                nc.tensor.matmul(zp[:], cmat(g["gi"]),
                                                 tvs[cp][:, :, st:64:2])
                                dst = zvs[cp][:, :, st:64:2]
                                ev.copy(dst, zp[:].rearrange(
                                    "p (b o) -> p b o", o=8))
                    zt = ztsb_p.tile([128, NSTR_CH * 256], F32R, tag="zt")
                    for jp in range(0, NSTR_CH, 2):
                        pb = ps_1b.tile([128, 512], F32, tag="pb", name="pb")
                        for k in range(4):
                            j = jp + k // 2
                            cp = k % 2
                            src = zsb[cp][:, j * 128:(j + 1) * 128]
                            nc.tensor.transpose(
                                pb[:, k * 128:(k + 1) * 128].bitcast(F32R),
                                src, ident)
                        ev.copy(zt[:, jp * 256:(jp + 2) * 256], pb[:])
                    for jq in range(0, NSTR_CH, 4):
                        ob = osb_p.tile([128, 1024], F32, tag="ob")
                        for h in range(2):
                            jp = jq + 2 * h
                            po = ps_1b.tile([128, 512], F32, tag="pb",
                                            name="po")
                            for k in range(2):
                                j = jp + k
                                nc.tensor.matmul(
                                    po[:, k * 256:(k + 1) * 256], LS4,
                                    zt[:, j * 256:(j + 1) * 256])
                            ev.copy(ob[:, h * 512:(h + 1) * 512], po[:])
                        nc.sync.dma_start(out[q, ch * 6 + jq // 4], ob[:])
    cap_sync_waits(nc)
    return nc


_CACHE = {}


def kernel(W, U_y, U_x, mask, block_rows, block_cols):
    from concourse import bass_utils
    enable_ldw_opt()

    W = np.ascontiguousarray(np.asarray(W, np.float32))
    const, ginfo = host_precompute(U_y, U_x, mask, block_rows, block_cols)
    n_mats = const.shape[1] // 128

    key = ("nc", n_mats, tuple(tuple(g["mem"]) for g in ginfo))
    if key not in _CACHE:
        _CACHE[key] = build_kernel(n_mats, ginfo)
    nc = _CACHE[key]

    in_maps = []
    for core in range(NCORE):
        Wsh = np.ascontiguousarray(W[:, core * CSH:(core + 1) * CSH])
        in_maps.append({"w": Wsh, "cst": const})

    res = None
    last_exc = None
    for attempt in range(3):
        try:
            res = bass_utils.run_bass_kernel_spmd(
                nc, in_maps, core_ids=list(range(NCORE)))
            break
        except Exception as e:  # transient NRT_EXEC_UNIT states recover
            last_exc = e
            time.sleep(20 * (attempt + 1))
    if res is None:
        raise last_exc
    outs = []
    for core in range(NCORE):
        o3 = res.results[core]["out"]          # [3, 12, 128, 1024]
        o = o3.reshape(3, 2, 6, 128, 4, 256).transpose(
            1, 2, 4, 3, 0, 5).reshape(6144, CSH)
        outs.append(o)
    return np.ascontiguousarray(np.concatenate(outs, axis=1))



# revision 13
# speedup vs baseline: 1.2963x; 1.2963x over previous
"""EquivariantProjectorViaSchur — TRN2 Bass kernel (8 NeuronCores, SPMD).

Math (per 64x64 channel block B of W):
    V   = U_y^T B U_x
    P   = A o V + Bc o V[sig_r][:, sig_c]     (= mask + gather-symmetrize-scatter)
    out = U_y P U_x^T
The masked symmetrization is fused into the PE matmuls via the k-group
structure of the Schur mask (8 rotation groups of 6, 2 parity groups of 8):
    Z[:, o in g]   = (s_g XG_g) @ T1T[:, o in g] + XJ_g @ T1s[:, o in g]
    XG_g = U_x diag(a_g) U_x^T        (symmetric; s_g = 1/2 rot, 1 diag)
    XJ_g[k',q] = 1/2 sum_{k in g} pi_k U_x[k',k] U_x[q, k^1]
    T1T  = (U_y^T B)^T  (produced directly by W-stationary matmuls)
    T1s[q,o] = pi_o * T1T[q, o^1]     (one DVE copy + one negate, strided APs)
    out  = kron(I2, U_y^T)-contraction of Z^T  (PE transpose + matmul)
Sharding: c_in block-columns — core i owns W[:, i*768:(i+1)*768]; the tiny
U/mask-derived factor matrices are replicated (precomputed host-side).
"""
import contextlib
import time

import numpy as np

import concourse.bass as bass
import concourse.tile as tile
import concourse.mybir as mybir
from concourse.tile import ScopedClock

F32 = mybir.dt.float32
F32R = mybir.dt.float32r
F16 = mybir.dt.float16

O = 64
NSTR_CH = 24          # 128-row stripes per b-chunk
NCH = 2               # b chunks of 48 blocks
NQ = 3                # c quads (4 c-blocks = 256 cols each)
NCORE = 8
CSH = 768             # columns per core shard


# ---------------------------------------------------------------------------
# workarounds for this toolchain
# ---------------------------------------------------------------------------
def _patched_drain_and_barrier(self, tick_clock, wait_clock):
    # this walrus build rejects >1 sem-wait on a Drain: split the tail waits
    drain_inst = self.nc.sync.drain()
    wait_clock.add_sem_waits(drain_inst.ins,
                             ScopedClock({None: tick_clock.global_clock}))
    si = drain_inst.ins.sync_info
    waits = list(si.on_wait) if si is not None else []
    if len(waits) > 1:
        drain_inst.ins.sync_info = mybir.SyncInfo(
            on_wait=waits[:1], on_update=list(si.on_update))
        for i in range(1, len(waits)):
            d2 = self.nc.sync.drain()
            d2.ins.sync_info = mybir.SyncInfo(on_wait=[waits[i]], on_update=[])
    self.nc.all_engine_barrier()
    assert self.sems is not None
    popped = self.nc._tile_sem_poison_stack.pop()
    assert popped is self._sem_poison
    self.nc.clear_and_free_semaphores(list(self.sems.allocated().values()))
    self.nc.all_engine_barrier()


tile.TileContext._drain_and_barrier = _patched_drain_and_barrier


def cap_sync_waits(nc):
    """walrus codegen allows only 1 sem-wait per instruction struct here;
    carry the excess on NoOps inserted just before (same engine/point)."""
    for f in nc.m.functions:
        for blk in f.blocks:
            insts = list(blk.instructions)
            out = []
            ctr = 0
            for ins in insts:
                si = ins.sync_info
                waits = list(si.on_wait) if si is not None else []
                if len(waits) > 1:
                    for i in range(len(waits) - 1):
                        n = mybir.InstNoOp(name=f"{ins.name}_w{ctr}",
                                           ins=[], outs=[])
                        ctr += 1
                        n.engine = ins.engine
                        n.sync_info = mybir.SyncInfo(on_wait=[waits[i]],
                                                     on_update=[])
                        out.append(n)
                    ins.sync_info = mybir.SyncInfo(
                        on_wait=waits[-1:], on_update=list(si.on_update))
                out.append(ins)
            blk.instructions = out


_LDW_PATCHED = False


def enable_ldw_opt():
    """--enable-ldw-opt=true: skip redundant LDWEIGHTS for runs of matmuls
    sharing the same stationary operand."""
    global _LDW_PATCHED
    if _LDW_PATCHED:
        return
    _LDW_PATCHED = True
    import concourse.bass_utils as bu
    orig = bu.bir_verify_and_optimise

    def patched(tmpdir, inp="bir.json", outp="file.neff", arch=None, *,
                dve_root=None):
        real_run = bu.run_command

        def run_hook(argv, **kw):
            argv = ["--enable-ldw-opt=true" if a == "--enable-ldw-opt=false"
                    else a for a in argv]
            return real_run(argv, **kw)
        bu.run_command = run_hook
        try:
            return orig(tmpdir, inp, outp, arch, dve_root=dve_root)
        finally:
            bu.run_command = real_run
    bu.bir_verify_and_optimise = patched


# ---------------------------------------------------------------------------
# host-side precompute of the replicated factor matrices
# ---------------------------------------------------------------------------
def host_precompute(U_y, U_x, mask, block_rows, block_cols):
    rows = np.asarray(block_rows); cols = np.asarray(block_cols)
    mask = np.asarray(mask)
    U_y64 = np.asarray(U_y, np.float64); U_x64 = np.asarray(U_x, np.float64)
    r_rot = set(int(x) for x in rows.tolist())
    nqd = len(rows) // 4
    for t in range(nqd):
        r = rows[4 * t:4 * t + 4]; c = cols[4 * t:4 * t + 4]
        assert mask[r, c].all()
        assert r[0] == r[1] and r[2] == r[3] and r[2] == r[0] + 1 and r[0] % 2 == 0
        assert c[0] == c[2] and c[1] == c[3] and c[1] == c[0] + 1 and c[0] % 2 == 0
    groups, seen = [], np.zeros(O, bool)
    for k in range(O):
        if seen[k]:
            continue
        mem = np.where(mask[k] > 0)[0]
        assert (mask[np.ix_(mem, mem)] > 0).all()
        for m in mem:
            seen[m] = True
        groups.append(mem)
    pi = np.where(np.arange(O) % 2 == 0, 1.0, -1.0)
    eye2 = np.eye(2)
    mats, ginfo = [], []
    mats.append(np.kron(eye2, U_y64).astype(np.float32))    # 0: LY (S1 moving)
    mats.append(np.kron(eye2, U_y64.T).astype(np.float32))  # 1: LS4
    mats.append(np.eye(128, dtype=np.float32))              # 2: identity
    for mem in groups:
        is_R = int(mem[0]) in r_rot
        s = 0.5 if is_R else 1.0
        a = np.zeros(O); a[mem] = 1.0
        XG = s * (U_x64 @ np.diag(a) @ U_x64.T)
        gi_idx = len(mats); mats.append(np.kron(eye2, XG).astype(np.float32))
        ji_idx = None
        if is_R:
            assert len(mem) == (mem[-1] - mem[0] + 1), "rot group not contiguous"
            XJ = np.zeros((O, O))
            for k in mem:
                XJ += 0.5 * pi[k] * np.outer(U_x64[:, k], U_x64[:, k ^ 1])
            ji_idx = len(mats); mats.append(np.kron(eye2, XJ.T).astype(np.float32))
        else:
            st = int(mem[0])
            assert all(int(m) == st + 2 * i for i, m in enumerate(mem)), \
                "diag group not stride-2"
        ginfo.append(dict(mem=[int(x) for x in mem], is_R=is_R,
                          gi=gi_idx, ji=ji_idx))
    const = np.concatenate(mats, axis=1)
    return np.ascontiguousarray(const.astype(np.float16)), ginfo


class _EvacBalancer:
    """Greedy ACT/DVE assignment for PSUM->SBUF copies. 16-bit src+dst
    with packed innermost APs hit the DVE 2x_1port mode (half cost)."""
    def __init__(self, nc):
        self.nc = nc
        self.t_act = 0.0
        self.t_dve = 0.0

    def copy(self, dst, src, both16=False):
        fd = src.free_size()
        c_act = (172.0 + fd) / 1.2
        c_dve = (120.0 + fd * (0.5 if both16 else 1.0)) / 0.96
        if self.t_act + c_act <= self.t_dve + c_dve:
            self.t_act += c_act
            return self.nc.scalar.copy(dst, src)
        else:
            self.t_dve += c_dve
            return self.nc.vector.tensor_copy(dst, src)


# ---------------------------------------------------------------------------
# device kernel (one program, SPMD over 8 cores)
# ---------------------------------------------------------------------------
def build_kernel(n_const_mats, ginfo):
    nc = bass.Bass("TRN2", target_bir_lowering=False, debug=False,
                   num_devices=1)
    w = nc.dram_tensor("w", [6144, CSH], F16, kind="ExternalInput").ap()
    cst = nc.dram_tensor("cst", [128, n_const_mats * 128], F16,
                         kind="ExternalInput").ap()
    out = nc.dram_tensor("out", [NQ, 12, 128, 1024], F16,
                         kind="ExternalOutput").ap()

    with tile.TileContext(nc) as tc:
        ctx = contextlib.ExitStack()
        with ctx:
            ev = _EvacBalancer(nc)
            csb_p = ctx.enter_context(tc.tile_pool(name="cst", bufs=1))
            wch_p = ctx.enter_context(tc.tile_pool(name="wch", bufs=10))
            t1T_p = ctx.enter_context(tc.tile_pool(name="t1T", bufs=1))
            t1s_p = ctx.enter_context(tc.tile_pool(name="t1s", bufs=1))
            zsb_p = ctx.enter_context(tc.tile_pool(name="zsb", bufs=1))
            ztsb_p = ctx.enter_context(tc.tile_pool(name="ztsb", bufs=1))
            osb_p = ctx.enter_context(tc.tile_pool(name="osb", bufs=6))
            ps_1b = ctx.enter_context(
                tc.tile_pool(name="ps_1b", bufs=8, space="PSUM"))

            csb = csb_p.tile([128, n_const_mats * 128], F16)
            nc.sync.dma_start(csb[:], cst[:])

            def cmat(i):
                return csb[:, i * 128:(i + 1) * 128]

            LY, LS4 = cmat(0), cmat(1)
            ident = cmat(2)

            for ch in range(NCH):
                # phase A: W-stationary S1' emits T1^T pieces directly
                t1Tq = [[t1T_p.tile([128, NSTR_CH * 128], F16,
                                    tag=f"t1T{q}_{cp}",
                                    name=f"t1T_{ch}_{q}_{cp}")
                         for cp in range(2)] for q in range(NQ)]
                for sg in range(0, NSTR_CH, 4):
                    grp = []
                    for k4 in range(4):
                        s = sg + k4
                        t = wch_p.tile([128, CSH], F16, tag="w",
                                       name=f"w_{ch}_{s}")
                        r0 = (ch * NSTR_CH + s) * 128
                        nc.sync.dma_start(t[:], w[r0:r0 + 128, :])
                        grp.append(t)
                    for q in range(NQ):
                        for cp in range(2):
                            pb = ps_1b.tile([128, 512], F32, tag="pb",
                                            name="pb")
                            for k in range(4):
                                lhsT = grp[k][:, (q * 2 + cp) * 128:
                                              (q * 2 + cp + 1) * 128]
                                nc.tensor.matmul(
                                    pb[:, k * 128:(k + 1) * 128], lhsT, LY)
                            ev.copy(t1Tq[q][cp][:, sg * 128:(sg + 4) * 128],
                                    pb[:])
                # phase B per quad: sigma prep, fused group matmuls, transpose,
                # final contraction, store
                for q in range(NQ):
                    t1T = t1Tq[q]
                    t1s = [t1s_p.tile([128, 48 * 48], F16, tag=f"t1s{cp}",
                                      name=f"t1s_{ch}_{q}_{cp}")
                           for cp in range(2)]
                    for cp in range(2):
                        tv = t1T[cp][:].rearrange("p (b o) -> p b o", o=64)
                        sv = t1s[cp][:].rearrange("p (b o) -> p b o", o=48)
                        nc.vector.tensor_copy(sv[:, :, 0:48:2],
                                              tv[:, :, 1:48:2])
                        nc.vector.tensor_scalar_mul(sv[:, :, 1:48:2],
                                                    tv[:, :, 0:48:2], -1.0)
                    zsb = [zsb_p.tile([128, NSTR_CH * 128], F16,
                                      tag=f"z{cp}", name=f"z_{ch}_{q}_{cp}")
                           for cp in range(2)]
                    tvs = [t1T[cp][:].rearrange("p (b o) -> p b o", o=64)
                           for cp in range(2)]
                    svs = [t1s[cp][:].rearrange("p (b o) -> p b o", o=48)
                           for cp in range(2)]
                    zvs = [zsb[cp][:].rearrange("p (b o) -> p b o", o=64)
                           for cp in range(2)]
                    for g in ginfo:
                        mem = g["mem"]
                        if g["is_R"]:
                            zps = []
                            for cp in range(2):
                                zp = ps_1b.tile([128, 48 * 6], F32, tag="pb",
                                                name="zp")
                                nc.tensor.matmul(
                                    zp[:], cmat(g["gi"]),
                                    tvs[cp][:, :, mem[0]:mem[0] + 6],
                                    start=True, stop=False)
                                zps.append(zp)
                            for cp in range(2):
                                nc.tensor.matmul(
                                    zps[cp][:], cmat(g["ji"]),
                                    svs[cp][:, :, mem[0]:mem[0] + 6],
                                    start=False, stop=True)
                                dst = zvs[cp][:, :, mem[0]:mem[0] + 6]
                                ev.copy(dst, zps[cp][:].rearrange(
                                    "p (b o) -> p b o", o=6))
                        else:
                            st = mem[0]
                            for cp in range(2):
                                zp = ps_1b.tile([128, 48 * 8], F32, tag="pb",
                                                name="zp")
                                nc.tensor.matmul(zp[:], cmat(g["gi"]),
                                                 tvs[cp][:, :, st:64:2])
                                dst = zvs[cp][:, :, st:64:2]
                                ev.copy(dst, zp[:].rearrange(
                                    "p (b o) -> p b o", o=8))
                    zt = ztsb_p.tile([128, NSTR_CH * 256], F16, tag="zt")
                    for jp in range(0, NSTR_CH, 4):
                        pb = ps_1b.tile([128, 1024], F16, tag="pb", name="pb")
                        for k in range(8):
                            j = jp + k // 2
                            cp = k % 2
                            src = zsb[cp][:, j * 128:(j + 1) * 128]
                            nc.tensor.transpose(
                                pb[:, k * 128:(k + 1) * 128], src, ident)
                        ev.copy(zt[:, jp * 256:(jp + 4) * 256], pb[:],
                                both16=True)
                    for jq in range(0, NSTR_CH, 4):
                        ob = osb_p.tile([128, 1024], F16, tag="ob")
                        for h in range(2):
                            jp = jq + 2 * h
                            po = ps_1b.tile([128, 512], F32, tag="pb",
                                            name="po")
                            nc.tensor.matmul(
                                po[:], LS4,
                                zt[:, jp * 256:(jp + 2) * 256])
                            ev.copy(ob[:, h * 512:(h + 1) * 512], po[:])
                        nc.sync.dma_start(out[q, ch * 6 + jq // 4], ob[:])
    cap_sync_waits(nc)
    return nc


_CACHE = {}


def kernel(W, U_y, U_x, mask, block_rows, block_cols):
    from concourse import bass_utils
    # ldw-opt rejects fp16-transpose LDWEIGHTS ("not compatible with LDW
    # optimization"); LDWEIGHTS pipelines behind matmuls, so leave it off.

    W = np.asarray(W, np.float32).astype(np.float16)
    const, ginfo = host_precompute(U_y, U_x, mask, block_rows, block_cols)
    n_mats = const.shape[1] // 128

    key = ("nc", n_mats, tuple(tuple(g["mem"]) for g in ginfo))
    if key not in _CACHE:
        _CACHE[key] = build_kernel(n_mats, ginfo)
    nc = _CACHE[key]

    in_maps = []
    for core in range(NCORE):
        Wsh = np.ascontiguousarray(W[:, core * CSH:(core + 1) * CSH])
        in_maps.append({"w": Wsh, "cst": const})

    res = None
    last_exc = None
    for attempt in range(3):
        try:
            res = bass_utils.run_bass_kernel_spmd(
                nc, in_maps, core_ids=list(range(NCORE)))
            break
        except Exception as e:  # transient NRT_EXEC_UNIT states recover
            last_exc = e
            time.sleep(20 * (attempt + 1))
    if res is None:
        raise last_exc
    outs = []
    for core in range(NCORE):
        o3 = np.asarray(res.results[core]["out"], np.float32)
        o = o3.reshape(3, 2, 6, 128, 4, 256).transpose(
            1, 2, 4, 3, 0, 5).reshape(6144, CSH)
        outs.append(o)
    return np.ascontiguousarray(np.concatenate(outs, axis=1))

